# revision 1
# baseline (speedup 1.0000x reference)
"""DilateAttention Trainium2 Bass kernel.

Problem: q,k,v [16, 128, 64, 64] f32; per-pixel attention over 9 dilated
(dil=2) 3x3 neighbors per head (4 heads x 32 dim); out [16, 64, 64, 128].

Sharding: data-parallel over batch B across 8 cores (2 images/core).

Layout: channel-major ([128 ch partitions, pixels free]). K and V are kept
as zero-padded 68x68 bf16 images so every shifted neighbor view is a
regular (dx, row, col) access pattern; the zero padding reproduces torch
Unfold semantics exactly, including the exp(0) softmax denominator terms
at borders.

Per 4-row chunk (256 px), kk grouped by dy (3 groups of 3), software
pipelined four chunks deep:
  products Q*K_kk (DVE bf16, one op per dy with broadcast-Q AP; the
  slack-tolerant tail planes run on the otherwise-idle GpSimd engine —
  DVE's bf16 tensor_tensor is 2x_1P single-port, so Pool's shared-port
  lock does not stall it) -> per-head sums via PE block-ones matmuls into
  grouped PSUM tiles -> exp on ACT (PSUM -> SBUF bf16, one op per group;
  run at full 128-partition extent so the per-head -> per-channel
  broadcast comes out of the exp for free) -> denominator via accumulated
  PE (block-ones/32) matmuls -> AV products (DVE + GpSimd tail) -> sum
  over kk via accumulated PE identity matmuls (one E plane pair is
  pre-added on Pool so the denominator accumulation streams 8 planes
  instead of 9) -> reciprocal_approx_fast + normalize (DVE) -> PE
  transpose to pixel-major -> ACT copy to SBUF -> output DMA batched
  over 2 chunks.

Engine busy (cost-model sim, per core): DVE 102us, ACT 101us, PE 100us,
Pool 100us; wall 127.3us (all four compute engines at ~80% utilization).
"""

import numpy as np
from contextlib import ExitStack

import concourse.bass as bass
import concourse.bacc as bacc
import concourse.tile as tile
from concourse import mybir
from concourse.bass_utils import run_bass_kernel_spmd
from concourse.masks import make_identity

F32 = mybir.dt.float32
BF16 = mybir.dt.bfloat16

B, D, H, W = 16, 128, 64, 64
NCORES = 8
BLOC = B // NCORES          # images per core
HEADS, HD = 4, 32
KS, DIL, PAD = 3, 2, 2
HP = H + 2 * PAD            # 68 (y-padded)
KK = KS * KS                # 9
SCALE = float(HD) ** -0.5
R = 4                       # image rows per chunk
NC = R * W                  # 256 pixels per chunk
NCHUNK = H // R             # 16
OGRP = 2                    # chunks batched per output DMA

PROFILE = False


WP2 = W + 2 * PAD  # 68 (x-padded too)


def _build_padded(nc, dst, stage, cast_engine):
    """dst: [128, HP, WP2] bf16 zero-padded image; stage: [128, H, W] f32."""
    nc.gpsimd.memset(dst[:, 0:PAD, :], 0.0)
    nc.gpsimd.memset(dst[:, HP - PAD:HP, :], 0.0)
    nc.gpsimd.memset(dst[:, PAD:HP - PAD, 0:PAD], 0.0)
    nc.gpsimd.memset(dst[:, PAD:HP - PAD, WP2 - PAD:WP2], 0.0)
    if cast_engine == "act":
        nc.scalar.copy(out=dst[:, PAD:HP - PAD, PAD:WP2 - PAD], in_=stage)
    elif cast_engine == "pool":
        nc.gpsimd.tensor_copy(out=dst[:, PAD:HP - PAD, PAD:WP2 - PAD],
                              in_=stage)
    else:
        nc.vector.tensor_copy(out=dst[:, PAD:HP - PAD, PAD:WP2 - PAD],
                              in_=stage)


def _bcast_q(qbf, y0):
    """[128, 3, R, W] view of qbf rows y0..y0+R with a 0-step dx axis."""
    base = qbf[:, y0:y0 + R, :]
    return bass.AP(
        tensor=base.tensor,
        offset=base.offset,
        ap=[list(base.ap[0]), [0, KS], [W, R], [1, W]],
    )


def _shift_view(pad_t, y0, idy, i0=0, n=KS):
    """[128, n(idx), R, W] view of padded image at dy=idy for chunk y0,
    idx range [i0, i0+n)."""
    return bass.AP(
        tensor=pad_t.tensor,
        offset=pad_t.offset + (y0 + DIL * idy) * WP2 + DIL * i0,
        ap=[list(pad_t.ap[0]), [DIL, n], [WP2, R], [1, W]],
    )


def _body(ctx: ExitStack, tc: tile.TileContext, out_ap, q_ap, k_ap, v_ap):
    nc = tc.nc

    consts = ctx.enter_context(tc.tile_pool(name="consts", bufs=1))
    stage_pool = ctx.enter_context(tc.tile_pool(name="stage_pool", bufs=3))
    perb = ctx.enter_context(tc.tile_pool(name="perb", bufs=2))
    work = ctx.enter_context(tc.tile_pool(name="work", bufs=8))
    egrp = ctx.enter_context(tc.tile_pool(name="egrp", bufs=14))
    outbuf = ctx.enter_context(tc.tile_pool(name="outbuf", bufs=4))
    psS = ctx.enter_context(tc.tile_pool(name="psS", bufs=2, space="PSUM"))
    psZ = ctx.enter_context(tc.tile_pool(name="psZ", bufs=2, space="PSUM"))
    psT = ctx.enter_context(tc.tile_pool(name="psT", bufs=2, space="PSUM"))

    # Constant stationary matrices.
    blockones = consts.tile([128, 128], BF16)   # 1 if same head
    bo32 = consts.tile([128, 128], BF16)        # 1/32 if same head
    identb = consts.tile([128, 128], BF16)
    identf = consts.tile([128, 128], F32)
    nc.vector.memset(blockones, 0.0)
    nc.vector.memset(bo32, 0.0)
    for h in range(HEADS):
        s = slice(h * HD, (h + 1) * HD)
        nc.vector.memset(blockones[s, s], 1.0)
        nc.vector.memset(bo32[s, s], 1.0 / HD)
    make_identity(nc, identb)
    make_identity(nc, identf)

    qf = q_ap.rearrange("b d h w -> b d (h w)")
    out_flat = out_ap.rearrange("b h w d -> b (h w) d")

    NQ = 4          # Q/K loads split into NQ interleaved row-band DMAs
    RB = H // NQ    # rows per band

    def load_b(b):
        qstage = stage_pool.tile([128, H, W], F32, name="qstage", tag="stage")
        kstage = stage_pool.tile([128, H, W], F32, name="kstage", tag="stage")
        qsrc = qf[b].rearrange("d (h w) -> d h w", w=W)
        ksrc = k_ap[b]
        for i in range(NQ):
            rs = slice(i * RB, (i + 1) * RB)
            nc.sync.dma_start(out=qstage[:, rs, :], in_=qsrc[:, rs, :])
            nc.sync.dma_start(out=kstage[:, rs, :], in_=ksrc[:, rs, :])
        qbf = perb.tile([128, H, W], BF16, name="qbf")
        kpad = perb.tile([128, HP, WP2], BF16, name="kpad")
        vpad = perb.tile([128, HP, WP2], BF16, name="vpad")
        nc.gpsimd.memset(kpad[:, 0:PAD, :], 0.0)
        nc.gpsimd.memset(kpad[:, HP - PAD:HP, :], 0.0)
        nc.gpsimd.memset(kpad[:, PAD:HP - PAD, 0:PAD], 0.0)
        nc.gpsimd.memset(kpad[:, PAD:HP - PAD, WP2 - PAD:WP2], 0.0)
        for i in range(NQ):
            rs = slice(i * RB, (i + 1) * RB)
            nc.scalar.copy(out=qbf[:, rs, :], in_=qstage[:, rs, :])
            nc.gpsimd.tensor_copy(
                out=kpad[:, PAD + i * RB:PAD + (i + 1) * RB, PAD:WP2 - PAD],
                in_=kstage[:, rs, :])
        stg = stage_pool.tile([128, H, W], F32, name="stg", tag="stage")
        nc.sync.dma_start(out=stg.rearrange("p h w -> p (h w)"),
                          in_=v_ap[b].rearrange("d h w -> d (h w)"))
        _build_padded(nc, vpad, stg, "pool")
        return qbf, kpad, vpad

    def stage_a(tens, ci):
        """QK products -> per-head score matmuls -> exp. Returns E groups."""
        qbf, kpad, vpad = tens
        y0 = ci * R
        qv = _bcast_q(qbf, y0)
        Eg = []
        for g in range(KS):  # g == idy
            Pg = egrp.tile([128, KS, R, W], BF16, name="Pg", tag="Pg")
            if g == KS - 1:
                nc.vector.tensor_mul(Pg[:, 0:2], qv[:, 0:2],
                                     _shift_view(kpad, y0, g, 0, 2))
                nc.gpsimd.tensor_mul(Pg[:, 2:3], qv[:, 2:3],
                                     _shift_view(kpad, y0, g, 2, 1))
            else:
                nc.vector.tensor_mul(Pg, qv, _shift_view(kpad, y0, g))
            Sgt = psS.tile([128, KS, R, W], F32, name="Sgt", tag="Sgt")
            # Planes 0-1 are one bank-aligned 2KB PSUM window: one N=512 MM.
            nc.tensor.matmul(Sgt[:, 0:2], blockones, Pg[:, 0:2],
                             start=True, stop=True)
            nc.tensor.matmul(Sgt[:, 2], blockones, Pg[:, 2],
                             start=True, stop=True)
            Egt = egrp.tile([128, KS, R, W], BF16, name="Egt", tag="Egt")
            nc.scalar.activation(out=Egt, in_=Sgt,
                                 func=mybir.ActivationFunctionType.Exp,
                                 scale=SCALE)
            Eg.append(Egt)
        return Eg

    state = {"outs": None}

    def stage_b1(tens, b, ci, Eg):
        """Z-sum and AV products."""
        qbf, kpad, vpad = tens
        y0 = ci * R
        zo = psZ.tile([128, 2, NC], F32, name="zo")
        Zp = zo[:, 0]
        # Pre-add one E plane pair on Pool (headroom there) so the PE
        # denominator accumulation streams 8 planes instead of 9; the pair
        # is accumulated last to give the Pool op maximum slack.
        Es01 = egrp.tile([128, R, W], BF16, name="Es01", tag="Es01")
        nc.gpsimd.tensor_add(Es01, Eg[0][:, 0], Eg[0][:, 1])
        for i, kk in enumerate(range(2, KK)):
            nc.tensor.matmul(Zp, bo32, Eg[kk // KS][:, kk % KS],
                             start=(i == 0), stop=False)
        nc.tensor.matmul(Zp, bo32, Es01, start=False, stop=True)
        P2g = []
        for g in range(KS):
            P2t = egrp.tile([128, KS, R, W], BF16, name="P2t", tag="P2t")
            if g == KS - 1:
                # Last group's tail planes are consumed last by the Oacc
                # accumulation; run them on the idle GpSimd engine (DVE's
                # bf16 TT is 2x_1P single-port, so no shared-port stall).
                nc.vector.tensor_mul(P2t[:, 0:1], Eg[g][:, 0:1],
                                     _shift_view(vpad, y0, g, 0, 1))
                nc.gpsimd.tensor_mul(P2t[:, 1:3], Eg[g][:, 1:3],
                                     _shift_view(vpad, y0, g, 1, 2))
            else:
                nc.vector.tensor_mul(P2t, Eg[g], _shift_view(vpad, y0, g))
            P2g.append(P2t)
        return zo, P2g

    def stage_b(tens, b, ci, bstate):
        """kk-sum, normalize, transpose, store."""
        zo, P2g = bstate
        Zp = zo[:, 0]
        Oacc = zo[:, 1]
        if ci % OGRP == 0:
            state["outs"] = outbuf.tile([128, OGRP, 2, 128], F32, name="outs")
        outs = state["outs"]
        for kk in range(KK):
            nc.tensor.matmul(Oacc, identb, P2g[kk // KS][:, kk % KS],
                             start=(kk == 0), stop=(kk == KK - 1))

        Rt = work.tile([128, NC], F32, name="Rt")
        nc.vector.reciprocal_approx_fast(out=Rt, in_=Zp)
        outn = work.tile([128, NC], F32, name="outn")
        nc.vector.tensor_mul(outn, Oacc, Rt)
        Tt = psT.tile([128, NC], F32, name="Tt")
        for j in range(2):
            nc.tensor.transpose(Tt[:, j * 128:(j + 1) * 128],
                                outn[:, j * 128:(j + 1) * 128], identf)
        nc.scalar.copy(out=outs[:, ci % OGRP],
                       in_=Tt.rearrange("p (j d) -> p j d", j=2))
        if ci % OGRP == OGRP - 1:
            c0 = ci - (OGRP - 1)
            dst = out_flat[b][c0 * NC:(ci + 1) * NC].rearrange(
                "(o j p) d -> p o j d", p=128, o=OGRP)
            nc.sync.dma_start(out=dst, in_=outs)

    # Software pipeline: stage_a of task i+1 is emitted before stage_b of
    # task i; b=1's loads are emitted right after the first chunk.
    DEPTH = 4
    tens = [load_b(b) for b in range(BLOC)]
    tasks = [(b, ci) for b in range(BLOC) for ci in range(NCHUNK)]
    pend = []
    for b, ci in tasks:
        Eg = stage_a(tens[b], ci)
        pend.append((b, ci, Eg))
        if len(pend) > DEPTH:
            pb, pci, pEg = pend.pop(0)
            stage_b(tens[pb], pb, pci, stage_b1(tens[pb], pb, pci, pEg))
    for pb, pci, pEg in pend:
        stage_b(tens[pb], pb, pci, stage_b1(tens[pb], pb, pci, pEg))


_CACHE = {}


def _build():
    if "nc" not in _CACHE:
        nc = bacc.Bacc("TRN2", target_bir_lowering=False, debug=False,
                       num_devices=NCORES)
        q = nc.dram_tensor("q", [BLOC, D, H, W], F32, kind="ExternalInput").ap()
        k = nc.dram_tensor("k", [BLOC, D, H, W], F32, kind="ExternalInput").ap()
        v = nc.dram_tensor("v", [BLOC, D, H, W], F32, kind="ExternalInput").ap()
        out = nc.dram_tensor("out", [BLOC, H, W, D], F32,
                             kind="ExternalOutput").ap()
        with tile.TileContext(nc) as tc:
            with ExitStack() as ctx:
                _body(ctx, tc, out, q, k, v)
        nc.compile()
        _CACHE["nc"] = nc
    return _CACHE["nc"]


def kernel(q, k, v):
    q = np.ascontiguousarray(np.asarray(q), dtype=np.float32)
    k = np.ascontiguousarray(np.asarray(k), dtype=np.float32)
    v = np.ascontiguousarray(np.asarray(v), dtype=np.float32)
    nc = _build()
    in_maps = [
        {
            "q": np.ascontiguousarray(q[i * BLOC:(i + 1) * BLOC]),
            "k": np.ascontiguousarray(k[i * BLOC:(i + 1) * BLOC]),
            "v": np.ascontiguousarray(v[i * BLOC:(i + 1) * BLOC]),
        }
        for i in range(NCORES)
    ]
    res = run_bass_kernel_spmd(nc, in_maps, list(range(NCORES)),
                               trace=PROFILE)
    out = np.concatenate([r["out"] for r in res.results], axis=0)
    if PROFILE:
        kernel.last_exec_time_ns = res.exec_time_ns
        kernel.last_results = res
    return out


if __name__ == "__main__":
    nc = _build()
    print("build OK")
    from concourse.timeline_sim import TimelineSim
    tl = TimelineSim(nc, trace=False)
    t = tl.simulate()
    print(f"TimelineSim: {t/1000.0:.1f} us")



# revision 41
# speedup vs baseline: 1.1000x; 1.1000x over previous
"""DilateAttention Trainium2 Bass kernel.

Problem: q,k,v [16, 128, 64, 64] f32; per-pixel attention over 9 dilated
(dil=2) 3x3 neighbors per head (4 heads x 32 dim); out [16, 64, 64, 128].

Sharding: data-parallel over batch B across 8 cores (2 images/core).

Layout: channel-major ([128 ch partitions, pixels free]). K and V are kept
as zero-padded 68x68 bf16 images so every shifted neighbor view is a
regular (dx, row, col) access pattern; the zero padding reproduces torch
Unfold semantics exactly, including the exp(0) softmax denominator terms
at borders.

Loads: gpsimd (SWDGE) DMAs cast f32->bf16 in flight (1 descriptor per
partition, ~1us Pool descriptor-gen per tensor) — no f32 staging and no
cast ops. k/v land in unpadded bf16 stages and are placed into the padded
images by 4x_2p DVE tensor_copy (0.26 ns/elem). A chain of dummy matmuls
keeps the PE continuously busy through the load phase so the cost model's
p-state ramp (2.4 GHz needs 3us of continuous execution) is already
satisfied when the first real matmul issues.

Per 4-row chunk (256 px), kk grouped by dy (3 groups of 3), software
pipelined four chunks deep:
  products Q*K_kk (DVE bf16 2x_1p; tail planes on GpSimd) -> per-head
  sums via PE block-ones matmuls into grouped PSUM tiles -> exp on ACT
  (PSUM -> SBUF bf16, one op per group; full 128-partition extent so the
  per-head -> per-channel broadcast is free) -> denominator via 9
  accumulated PE (block-ones/32) matmuls -> AV products (DVE + GpSimd
  tail, alternating split to balance) -> sum over kk via accumulated PE
  identity matmuls -> reciprocal_approx_fast + normalize to bf16 (DVE)
  -> PE bf16 transpose (1 cyc/row) to pixel-major -> ACT copy casts to
  f32 SBUF -> per-chunk output DMA.

Engine busy (cost-model sim, per core): DVE ~99us, PE ~96us, ACT ~93us,
Pool ~92us, DMA device ~39us; wall 115.7us.
"""

import numpy as np
from contextlib import ExitStack

import concourse.bass as bass
import concourse.bacc as bacc
import concourse.tile as tile
from concourse import mybir
from concourse.bass_utils import run_bass_kernel_spmd
from concourse.masks import make_identity

F32 = mybir.dt.float32
BF16 = mybir.dt.bfloat16

B, D, H, W = 16, 128, 64, 64
NCORES = 8
BLOC = B // NCORES          # images per core
HEADS, HD = 4, 32
KS, DIL, PAD = 3, 2, 2
HP = H + 2 * PAD            # 68 (y-padded)
KK = KS * KS                # 9
SCALE = float(HD) ** -0.5
R = 4                       # image rows per chunk
NC = R * W                  # 256 pixels per chunk
NCHUNK = H // R             # 16
OGRP = 1                    # chunks batched per output DMA

PROFILE = False


WP2 = W + 2 * PAD  # 68 (x-padded too)


def _build_padded(nc, dst, stage, cast_engine):
    """dst: [128, HP, WP2] bf16 zero-padded image; stage: [128, H, W] f32."""
    nc.gpsimd.memset(dst[:, 0:PAD, :], 0.0)
    nc.gpsimd.memset(dst[:, HP - PAD:HP, :], 0.0)
    nc.gpsimd.memset(dst[:, PAD:HP - PAD, 0:PAD], 0.0)
    nc.gpsimd.memset(dst[:, PAD:HP - PAD, WP2 - PAD:WP2], 0.0)
    if cast_engine == "act":
        nc.scalar.copy(out=dst[:, PAD:HP - PAD, PAD:WP2 - PAD], in_=stage)
    elif cast_engine == "pool":
        nc.gpsimd.tensor_copy(out=dst[:, PAD:HP - PAD, PAD:WP2 - PAD],
                              in_=stage)
    else:
        nc.vector.tensor_copy(out=dst[:, PAD:HP - PAD, PAD:WP2 - PAD],
                              in_=stage)


USE_STT = False


def _pool_mul(nc, out, a, b):
    """Pool-engine multiply via scalar_tensor_tensor: the TensorScalarPtr
    opcode takes the 0.60 gpsimd efficiency bucket instead of
    tensor_tensor's 0.42 Multiply bucket."""
    if USE_STT:
        nc.gpsimd.scalar_tensor_tensor(out, a, 1.0, b,
                                       op0=mybir.AluOpType.mult,
                                       op1=mybir.AluOpType.mult)
    else:
        nc.gpsimd.tensor_mul(out, a, b)


def _pool_add(nc, out, a, b):
    if USE_STT:
        nc.gpsimd.scalar_tensor_tensor(out, a, 1.0, b,
                                       op0=mybir.AluOpType.mult,
                                       op1=mybir.AluOpType.add)
    else:
        nc.gpsimd.tensor_add(out, a, b)


def _bcast_q(qbf, y0):
    """[128, 3, R, W] view of qbf rows y0..y0+R with a 0-step dx axis."""
    base = qbf[:, y0:y0 + R, :]
    return bass.AP(
        tensor=base.tensor,
        offset=base.offset,
        ap=[list(base.ap[0]), [0, KS], [W, R], [1, W]],
    )


def _shift_view(pad_t, y0, idy, i0=0, n=KS):
    """[128, n(idx), R, W] view of padded image at dy=idy for chunk y0,
    idx range [i0, i0+n)."""
    return bass.AP(
        tensor=pad_t.tensor,
        offset=pad_t.offset + (y0 + DIL * idy) * WP2 + DIL * i0,
        ap=[list(pad_t.ap[0]), [DIL, n], [WP2, R], [1, W]],
    )


def _body(ctx: ExitStack, tc: tile.TileContext, out_ap, q_ap, k_ap, v_ap):
    nc = tc.nc

    consts = ctx.enter_context(tc.tile_pool(name="consts", bufs=1))
    stage_pool = ctx.enter_context(tc.tile_pool(name="stage_pool", bufs=3))
    perb = ctx.enter_context(tc.tile_pool(name="perb", bufs=2))
    work = ctx.enter_context(tc.tile_pool(name="work", bufs=8))
    egrp = ctx.enter_context(tc.tile_pool(name="egrp", bufs=14))
    outbuf = ctx.enter_context(tc.tile_pool(name="outbuf", bufs=4))
    psS = ctx.enter_context(tc.tile_pool(name="psS", bufs=2, space="PSUM"))
    psZ = ctx.enter_context(tc.tile_pool(name="psZ", bufs=2, space="PSUM"))
    psT = ctx.enter_context(tc.tile_pool(name="psT", bufs=2, space="PSUM"))

    # Constant stationary matrices.
    blockones = consts.tile([128, 128], BF16)   # 1 if same head
    bo32 = consts.tile([128, 128], BF16)        # 1/32 if same head
    identb = consts.tile([128, 128], BF16)
    identf = consts.tile([128, 128], F32)
    nc.vector.memset(blockones, 0.0)
    nc.vector.memset(bo32, 0.0)
    for h in range(HEADS):
        s = slice(h * HD, (h + 1) * HD)
        nc.vector.memset(blockones[s, s], 1.0)
        nc.vector.memset(bo32[s, s], 1.0 / HD)
    make_identity(nc, identb)
    make_identity(nc, identf)

    qf = q_ap.rearrange("b d h w -> b d (h w)")
    out_flat = out_ap.rearrange("b h w d -> b (h w) d")

    NQ = 2          # pad-copy row bands per image
    RB = H // NQ    # rows per band

    def load_b(b):
        # gpsimd (SWDGE) DMAs cast f32->bf16 in flight: one descriptor per
        # partition, ~1us of Pool descriptor-gen per tensor, and no f32
        # staging or cast ops at all. k/v land in unpadded bf16 stages and
        # are placed into the zero-padded images by 4x_2p DVE copies.
        # For b=0 the q/k transfers are split and interleaved so the first
        # chunks' working set lands in ~4us instead of ~7.
        qbf = perb.tile([128, H, W], BF16, name="qbf")
        kst = stage_pool.tile([128, H, W], BF16, name="kst", tag="stage")
        vst = stage_pool.tile([128, H, W], BF16, name="vst", tag="stage")
        kpad = perb.tile([128, HP, WP2], BF16, name="kpad")
        vpad = perb.tile([128, HP, WP2], BF16, name="vpad")
        # Border memsets first: they depend on nothing, so Pool clears them
        # before the SWDGE descriptor-generation ops queue up behind them.
        for pad_t in (kpad, vpad):
            nc.gpsimd.memset(pad_t[:, 0:PAD, :], 0.0)
            nc.gpsimd.memset(pad_t[:, HP - PAD:HP, :], 0.0)
            nc.gpsimd.memset(pad_t[:, PAD:HP - PAD, 0:PAD], 0.0)
            nc.gpsimd.memset(pad_t[:, PAD:HP - PAD, WP2 - PAD:WP2], 0.0)
        qsrc = qf[b].rearrange("d (h w) -> d h w", w=W)
        if b == 0:
            hh = H // 2
            nc.gpsimd.dma_start(out=qbf[:, 0:hh], in_=qsrc[:, 0:hh])
            nc.gpsimd.dma_start(out=kst[:, 0:hh], in_=k_ap[b][:, 0:hh])
            nc.gpsimd.dma_start(out=qbf[:, hh:H], in_=qsrc[:, hh:H])
            nc.gpsimd.dma_start(out=kst[:, hh:H], in_=k_ap[b][:, hh:H])
        else:
            nc.gpsimd.dma_start(out=qbf, in_=qsrc)
            nc.gpsimd.dma_start(out=kst, in_=k_ap[b])
        nc.gpsimd.dma_start(out=vst, in_=v_ap[b])
        for pad_t, st in ((kpad, kst), (vpad, vst)):
            for i in range(NQ):
                rs = slice(i * RB, (i + 1) * RB)
                ps = slice(PAD + i * RB, PAD + (i + 1) * RB)
                nc.vector.tensor_copy(out=pad_t[:, ps, PAD:WP2 - PAD],
                                      in_=st[:, rs, :])
        return qbf, kpad, vpad

    def stage_a(tens, ci):
        """QK products -> per-head score matmuls -> exp. Returns E groups."""
        qbf, kpad, vpad = tens
        y0 = ci * R
        qv = _bcast_q(qbf, y0)
        Eg = []
        for g in range(KS):  # g == idy
            Pg = egrp.tile([128, KS, R, W], BF16, name="Pg", tag="Pg")
            if g == KS - 1:
                nc.vector.tensor_mul(Pg[:, 0:1], qv[:, 0:1],
                                     _shift_view(kpad, y0, g, 0, 1))
                _pool_mul(nc, Pg[:, 1:3], qv[:, 1:3],
                          _shift_view(kpad, y0, g, 1, 2))
            else:
                nc.vector.tensor_mul(Pg, qv, _shift_view(kpad, y0, g))
            Sgt = psS.tile([128, KS, R, W], F32, name="Sgt", tag="Sgt")
            # Planes 0-1 are one bank-aligned 2KB PSUM window: one N=512 MM.
            nc.tensor.matmul(Sgt[:, 0:2], blockones, Pg[:, 0:2],
                             start=True, stop=True)
            nc.tensor.matmul(Sgt[:, 2], blockones, Pg[:, 2],
                             start=True, stop=True)
            Egt = egrp.tile([128, KS, R, W], BF16, name="Egt", tag="Egt")
            nc.scalar.activation(out=Egt, in_=Sgt,
                                 func=mybir.ActivationFunctionType.Exp,
                                 scale=SCALE)
            Eg.append(Egt)
        return Eg

    state = {"outs": None}

    def stage_b1(tens, b, ci, Eg):
        """Z-sum and AV products."""
        qbf, kpad, vpad = tens
        y0 = ci * R
        fold = ci % 2 == 0     # alternate one AV plane DVE<->Pool
        zo = psZ.tile([128, 2, NC], F32, name="zo")
        Zp = zo[:, 0]
        for kk in range(KK):
            nc.tensor.matmul(Zp, bo32, Eg[kk // KS][:, kk % KS],
                             start=(kk == 0), stop=(kk == KK - 1))
        P2g = []
        for g in range(KS):
            P2t = egrp.tile([128, KS, R, W], BF16, name="P2t", tag="P2t")
            if g == KS - 1:
                # Last group's tail planes are consumed last by the Oacc
                # accumulation; run them on the GpSimd engine. On alternate
                # chunks the whole group goes to GpSimd to balance DVE.
                if fold:
                    _pool_mul(nc, P2t, Eg[g], _shift_view(vpad, y0, g))
                else:
                    nc.vector.tensor_mul(P2t[:, 0:1], Eg[g][:, 0:1],
                                         _shift_view(vpad, y0, g, 0, 1))
                    _pool_mul(nc, P2t[:, 1:3], Eg[g][:, 1:3],
                              _shift_view(vpad, y0, g, 1, 2))
            else:
                nc.vector.tensor_mul(P2t, Eg[g], _shift_view(vpad, y0, g))
            P2g.append(P2t)
        return zo, P2g

    def stage_b(tens, b, ci, bstate):
        """kk-sum, normalize, transpose, store."""
        zo, P2g = bstate
        Zp = zo[:, 0]
        Oacc = zo[:, 1]
        if ci % OGRP == 0:
            state["outs"] = outbuf.tile([128, OGRP, 2, 128], F32, name="outs")
        outs = state["outs"]
        for kk in range(KK):
            nc.tensor.matmul(Oacc, identb, P2g[kk // KS][:, kk % KS],
                             start=(kk == 0), stop=(kk == KK - 1))

        Rt = work.tile([128, NC], F32, name="Rt")
        nc.vector.reciprocal_approx_fast(out=Rt, in_=Zp)
        outn = work.tile([128, NC], BF16, name="outn")
        nc.vector.tensor_mul(outn, Oacc, Rt)
        Tt = psT.tile([128, NC], BF16, name="Tt")
        for j in range(2):
            nc.tensor.transpose(Tt[:, j * 128:(j + 1) * 128],
                                outn[:, j * 128:(j + 1) * 128], identb)
        nc.scalar.copy(out=outs[:, ci % OGRP],
                       in_=Tt.rearrange("p (j d) -> p j d", j=2))
        if ci % OGRP == OGRP - 1:
            c0 = ci - (OGRP - 1)
            dst = out_flat[b][c0 * NC:(ci + 1) * NC].rearrange(
                "(o j p) d -> p o j d", p=128, o=OGRP)
            nc.sync.dma_start(out=dst, in_=outs)

    # PE warmup: a chain of dependent matmuls into one scratch PSUM tile
    # keeps the Tensor engine continuously busy through the load phase, so
    # the cost model's p-state ramp (2.4 GHz only after 3us of continuous
    # execution) is already satisfied when the first real matmul issues.
    # The warmup tiles cycle psZ's slots (same 2KB size, same untagged tag)
    # so no extra PSUM bank is needed.
    WARMUP = 24
    ident4 = bass.AP(tensor=identb.tensor, offset=identb.offset,
                     ap=[list(identb.ap[0]), [0, 4], [1, 128]])
    for _ in range(WARMUP):
        warm = psZ.tile([128, 2, NC], F32, name="warm", tag="zo")
        nc.tensor.matmul(warm.rearrange("p a b -> p (a b)"), blockones,
                         ident4, start=True, stop=True)

    # Software pipeline: stage_a of task i+1 is emitted before stage_b of
    # task i; b=1's loads are emitted a few chunks into b=0's stream so
    # their Pool-side descriptor generation doesn't compete at startup.
    DEPTH = 4
    tens = [load_b(0), None]
    tasks = [(b, ci) for b in range(BLOC) for ci in range(NCHUNK)]
    pend = []
    ntask = len(tasks)
    for ti, (b, ci) in enumerate(tasks):
        if ti == 14:
            tens[1] = load_b(1)
        Eg = stage_a(tens[b], ci)
        pend.append((b, ci, Eg))
        depth_now = DEPTH
        while len(pend) > depth_now:
            pb, pci, pEg = pend.pop(0)
            stage_b(tens[pb], pb, pci, stage_b1(tens[pb], pb, pci, pEg))
    for pb, pci, pEg in pend:
        stage_b(tens[pb], pb, pci, stage_b1(tens[pb], pb, pci, pEg))


_CACHE = {}


def _build():
    if "nc" not in _CACHE:
        nc = bacc.Bacc("TRN2", target_bir_lowering=False, debug=False,
                       num_devices=NCORES)
        q = nc.dram_tensor("q", [BLOC, D, H, W], F32, kind="ExternalInput").ap()
        k = nc.dram_tensor("k", [BLOC, D, H, W], F32, kind="ExternalInput").ap()
        v = nc.dram_tensor("v", [BLOC, D, H, W], F32, kind="ExternalInput").ap()
        out = nc.dram_tensor("out", [BLOC, H, W, D], F32,
                             kind="ExternalOutput").ap()
        with tile.TileContext(nc) as tc:
            with ExitStack() as ctx:
                _body(ctx, tc, out, q, k, v)
        nc.compile()
        _CACHE["nc"] = nc
    return _CACHE["nc"]


def kernel(q, k, v):
    q = np.ascontiguousarray(np.asarray(q), dtype=np.float32)
    k = np.ascontiguousarray(np.asarray(k), dtype=np.float32)
    v = np.ascontiguousarray(np.asarray(v), dtype=np.float32)
    nc = _build()
    in_maps = [
        {
            "q": np.ascontiguousarray(q[i * BLOC:(i + 1) * BLOC]),
            "k": np.ascontiguousarray(k[i * BLOC:(i + 1) * BLOC]),
            "v": np.ascontiguousarray(v[i * BLOC:(i + 1) * BLOC]),
        }
        for i in range(NCORES)
    ]
    res = run_bass_kernel_spmd(nc, in_maps, list(range(NCORES)),
                               trace=PROFILE)
    out = np.concatenate([r["out"] for r in res.results], axis=0)
    if PROFILE:
        kernel.last_exec_time_ns = res.exec_time_ns
        kernel.last_results = res
    return out


if __name__ == "__main__":
    nc = _build()
    print("build OK")
    from concourse.timeline_sim import TimelineSim
    tl = TimelineSim(nc, trace=False)
    t = tl.simulate()
    print(f"TimelineSim: {t/1000.0:.1f} us")



# revision 63
# speedup vs baseline: 1.1038x; 1.0034x over previous
"""DilateAttention Trainium2 Bass kernel.

Problem: q,k,v [16, 128, 64, 64] f32; per-pixel attention over 9 dilated
(dil=2) 3x3 neighbors per head (4 heads x 32 dim); out [16, 64, 64, 128].

Sharding: data-parallel over batch B across 8 cores (2 images/core).

Layout: channel-major ([128 ch partitions, pixels free]). K and V are kept
as zero-padded 68x68 bf16 images so every shifted neighbor view is a
regular (dx, row, col) access pattern; the zero padding reproduces torch
Unfold semantics exactly, including the exp(0) softmax denominator terms
at borders.

Loads: gpsimd (SWDGE) DMAs cast f32->bf16 in flight (1 descriptor per
partition, ~1us Pool descriptor-gen per tensor) — no f32 staging and no
cast ops. k/v land in unpadded bf16 stages and are placed into the padded
images by 4x_2p DVE tensor_copy (0.26 ns/elem). A chain of dummy matmuls
keeps the PE continuously busy through the load phase so the cost model's
p-state ramp (2.4 GHz needs 3us of continuous execution) is already
satisfied when the first real matmul issues.

Per 4-row chunk (256 px), kk grouped by dy (3 groups of 3), software
pipelined four chunks deep:
  products Q*K_kk (DVE bf16 2x_1p; tail planes on GpSimd) -> per-head
  sums via PE block-ones matmuls into grouped PSUM tiles -> exp on ACT
  (PSUM -> SBUF bf16, one op per group; full 128-partition extent so the
  per-head -> per-channel broadcast is free) -> denominator via 9
  accumulated PE (block-ones/32) matmuls -> AV products (DVE + GpSimd
  tail, alternating split to balance) -> sum over kk via accumulated PE
  identity matmuls -> reciprocal_approx_fast + normalize to bf16 (DVE)
  -> PE bf16 transpose (1 cyc/row) to pixel-major -> ACT copy casts to
  f32 SBUF -> per-chunk output DMA.

Engine busy (cost-model sim, per core): PE ~96us (the steady-state bound:
scores 960 + Z 960 + O 960 + transpose 107 ns per 256-px chunk), DVE
~99us, ACT ~93us, Pool ~92us; wall 115.7us = PE + ~12us startup + ~6us
drain. (5-dim-AP merged product ops shaved this to 115.5 in the cost
model but the walrus backend rejects them, so per-dy ops stand.)
"""

import numpy as np
from contextlib import ExitStack

import concourse.bass as bass
import concourse.bacc as bacc
import concourse.tile as tile
from concourse import mybir
from concourse.bass_utils import run_bass_kernel_spmd
from concourse.masks import make_identity

F32 = mybir.dt.float32
BF16 = mybir.dt.bfloat16

B, D, H, W = 16, 128, 64, 64
NCORES = 8
BLOC = B // NCORES          # images per core
HEADS, HD = 4, 32
KS, DIL, PAD = 3, 2, 2
HP = H + 2 * PAD            # 68 (y-padded)
KK = KS * KS                # 9
SCALE = float(HD) ** -0.5
R = 4                       # image rows per chunk
NC = R * W                  # 256 pixels per chunk
NCHUNK = H // R             # 16
OGRP = 1                    # chunks batched per output DMA

PROFILE = False


WP2 = W + 2 * PAD  # 68 (x-padded too)


def _build_padded(nc, dst, stage, cast_engine):
    """dst: [128, HP, WP2] bf16 zero-padded image; stage: [128, H, W] f32."""
    nc.gpsimd.memset(dst[:, 0:PAD, :], 0.0)
    nc.gpsimd.memset(dst[:, HP - PAD:HP, :], 0.0)
    nc.gpsimd.memset(dst[:, PAD:HP - PAD, 0:PAD], 0.0)
    nc.gpsimd.memset(dst[:, PAD:HP - PAD, WP2 - PAD:WP2], 0.0)
    if cast_engine == "act":
        nc.scalar.copy(out=dst[:, PAD:HP - PAD, PAD:WP2 - PAD], in_=stage)
    elif cast_engine == "pool":
        nc.gpsimd.tensor_copy(out=dst[:, PAD:HP - PAD, PAD:WP2 - PAD],
                              in_=stage)
    else:
        nc.vector.tensor_copy(out=dst[:, PAD:HP - PAD, PAD:WP2 - PAD],
                              in_=stage)


USE_STT = False


def _pool_mul(nc, out, a, b):
    """Pool-engine multiply via scalar_tensor_tensor: the TensorScalarPtr
    opcode takes the 0.60 gpsimd efficiency bucket instead of
    tensor_tensor's 0.42 Multiply bucket."""
    if USE_STT:
        nc.gpsimd.scalar_tensor_tensor(out, a, 1.0, b,
                                       op0=mybir.AluOpType.mult,
                                       op1=mybir.AluOpType.mult)
    else:
        nc.gpsimd.tensor_mul(out, a, b)


def _pool_add(nc, out, a, b):
    if USE_STT:
        nc.gpsimd.scalar_tensor_tensor(out, a, 1.0, b,
                                       op0=mybir.AluOpType.mult,
                                       op1=mybir.AluOpType.add)
    else:
        nc.gpsimd.tensor_add(out, a, b)


def _bcast_q(qbf, y0, rows=R):
    """[128, 3, rows, W] view of qbf rows y0.. with a 0-step dx axis."""
    base = qbf[:, y0:y0 + rows, :]
    return bass.AP(
        tensor=base.tensor,
        offset=base.offset,
        ap=[list(base.ap[0]), [0, KS], [W, rows], [1, W]],
    )


def _shift_view(pad_t, y0, idy, i0=0, n=KS, rows=R):
    """[128, n(idx), rows, W] view of padded image at dy=idy for chunk y0,
    idx range [i0, i0+n)."""
    return bass.AP(
        tensor=pad_t.tensor,
        offset=pad_t.offset + (y0 + DIL * idy) * WP2 + DIL * i0,
        ap=[list(pad_t.ap[0]), [DIL, n], [WP2, rows], [1, W]],
    )


def _body(ctx: ExitStack, tc: tile.TileContext, out_ap, q_ap, k_ap, v_ap):
    nc = tc.nc

    consts = ctx.enter_context(tc.tile_pool(name="consts", bufs=1))
    stage_pool = ctx.enter_context(tc.tile_pool(name="stage_pool", bufs=3))
    perb = ctx.enter_context(tc.tile_pool(name="perb", bufs=2))
    work = ctx.enter_context(tc.tile_pool(name="work", bufs=8))
    egrp = ctx.enter_context(tc.tile_pool(name="egrp", bufs=14))
    outbuf = ctx.enter_context(tc.tile_pool(name="outbuf", bufs=4))
    psS = ctx.enter_context(tc.tile_pool(name="psS", bufs=2, space="PSUM"))
    psZ = ctx.enter_context(tc.tile_pool(name="psZ", bufs=2, space="PSUM"))
    psT = ctx.enter_context(tc.tile_pool(name="psT", bufs=2, space="PSUM"))

    # Constant stationary matrices.
    blockones = consts.tile([128, 128], BF16)   # 1 if same head
    bo32 = consts.tile([128, 128], BF16)        # 1/32 if same head
    identb = consts.tile([128, 128], BF16)
    identf = consts.tile([128, 128], F32)
    nc.vector.memset(blockones, 0.0)
    nc.vector.memset(bo32, 0.0)
    for h in range(HEADS):
        s = slice(h * HD, (h + 1) * HD)
        nc.vector.memset(blockones[s, s], 1.0)
        nc.vector.memset(bo32[s, s], 1.0 / HD)
    make_identity(nc, identb)
    make_identity(nc, identf)

    qf = q_ap.rearrange("b d h w -> b d (h w)")
    out_flat = out_ap.rearrange("b h w d -> b (h w) d")

    NQ = 2          # pad-copy row bands per image
    RB = H // NQ    # rows per band

    def load_b(b):
        # gpsimd (SWDGE) DMAs cast f32->bf16 in flight: one descriptor per
        # partition, ~1us of Pool descriptor-gen per tensor, and no f32
        # staging or cast ops at all. k/v land in unpadded bf16 stages and
        # are placed into the zero-padded images by 4x_2p DVE copies.
        # For b=0 the q/k transfers are split and interleaved so the first
        # chunks' working set lands in ~4us instead of ~7.
        qbf = perb.tile([128, H, W], BF16, name="qbf")
        kst = stage_pool.tile([128, H, W], BF16, name="kst", tag="stage")
        vst = stage_pool.tile([128, H, W], BF16, name="vst", tag="stage")
        kpad = perb.tile([128, HP, WP2], BF16, name="kpad")
        vpad = perb.tile([128, HP, WP2], BF16, name="vpad")
        # For b=0, the first q/k half-gens go ahead of everything so their
        # transfers start immediately; the memsets fill Pool's time while
        # those transfers are in flight, then the remaining gens follow.
        qsrc0 = qf[b].rearrange("d (h w) -> d h w", w=W)
        if b == 0:
            nc.gpsimd.dma_start(out=qbf[:, 0:H // 2], in_=qsrc0[:, 0:H // 2])
            nc.gpsimd.dma_start(out=kst[:, 0:H // 2],
                                in_=k_ap[b][:, 0:H // 2])
        for pad_t in (kpad, vpad):
            nc.gpsimd.memset(pad_t[:, 0:PAD, :], 0.0)
            nc.gpsimd.memset(pad_t[:, HP - PAD:HP, :], 0.0)
            nc.gpsimd.memset(pad_t[:, PAD:HP - PAD, 0:PAD], 0.0)
            nc.gpsimd.memset(pad_t[:, PAD:HP - PAD, WP2 - PAD:WP2], 0.0)
        qsrc = qf[b].rearrange("d (h w) -> d h w", w=W)
        if b == 0:
            hh = H // 2
            nc.gpsimd.dma_start(out=qbf[:, hh:H], in_=qsrc[:, hh:H])
            nc.gpsimd.dma_start(out=kst[:, hh:H], in_=k_ap[b][:, hh:H])
        else:
            nc.gpsimd.dma_start(out=qbf, in_=qsrc)
            nc.gpsimd.dma_start(out=kst, in_=k_ap[b])
        nc.gpsimd.dma_start(out=vst, in_=v_ap[b])
        for pad_t, st in ((kpad, kst), (vpad, vst)):
            for i in range(NQ):
                rs = slice(i * RB, (i + 1) * RB)
                ps = slice(PAD + i * RB, PAD + (i + 1) * RB)
                nc.vector.tensor_copy(out=pad_t[:, ps, PAD:WP2 - PAD],
                                      in_=st[:, rs, :])
        return qbf, kpad, vpad

    def stage_a(tens, y0, rows):
        """QK products -> per-head score matmuls -> exp. Returns E groups."""
        qbf, kpad, vpad = tens
        qv = _bcast_q(qbf, y0, rows)
        Eg = []
        for g in range(KS):  # g == idy
            Pg = egrp.tile([128, KS, rows, W], BF16, name="Pg", tag="Pg")
            if g == KS - 1:
                nc.vector.tensor_mul(Pg[:, 0:1], qv[:, 0:1],
                                     _shift_view(kpad, y0, g, 0, 1, rows))
                _pool_mul(nc, Pg[:, 1:3], qv[:, 1:3],
                          _shift_view(kpad, y0, g, 1, 2, rows))
            else:
                nc.vector.tensor_mul(Pg, qv, _shift_view(kpad, y0, g,
                                                         rows=rows))
            Sgt = psS.tile([128, KS, rows, W], F32, name="Sgt", tag="Sgt")
            # Planes 0-1 are one bank-aligned 2KB PSUM window: one N=512 MM.
            nc.tensor.matmul(Sgt[:, 0:2], blockones, Pg[:, 0:2],
                             start=True, stop=True)
            nc.tensor.matmul(Sgt[:, 2], blockones, Pg[:, 2],
                             start=True, stop=True)
            Egt = egrp.tile([128, KS, rows, W], BF16, name="Egt", tag="Egt")
            nc.scalar.activation(out=Egt, in_=Sgt,
                                 func=mybir.ActivationFunctionType.Exp,
                                 scale=SCALE)
            Eg.append(Egt)
        return Eg

    state = {"outs": None}

    def stage_b1(tens, y0, rows, fold, Eg):
        """Z-sum and AV products."""
        qbf, kpad, vpad = tens
        zo = psZ.tile([128, 2, rows * W], F32, name="zo", tag="zo")
        Zp = zo[:, 0]
        for kk in range(KK):
            nc.tensor.matmul(Zp, bo32, Eg[kk // KS][:, kk % KS],
                             start=(kk == 0), stop=(kk == KK - 1))
        P2g = []
        for g in range(KS):
            P2t = egrp.tile([128, KS, rows, W], BF16, name="P2t", tag="P2t")
            if g == KS - 1:
                # Last group's tail planes are consumed last by the Oacc
                # accumulation; run them on the GpSimd engine. On alternate
                # chunks the whole group goes to GpSimd to balance DVE.
                if fold:
                    _pool_mul(nc, P2t, Eg[g],
                              _shift_view(vpad, y0, g, rows=rows))
                else:
                    nc.vector.tensor_mul(P2t[:, 0:1], Eg[g][:, 0:1],
                                         _shift_view(vpad, y0, g, 0, 1,
                                                     rows))
                    _pool_mul(nc, P2t[:, 1:3], Eg[g][:, 1:3],
                              _shift_view(vpad, y0, g, 1, 2, rows))
            else:
                nc.vector.tensor_mul(P2t, Eg[g],
                                     _shift_view(vpad, y0, g, rows=rows))
            P2g.append(P2t)
        return zo, P2g

    def stage_b(tens, b, y0, rows, bstate):
        """kk-sum, normalize, transpose, store."""
        zo, P2g = bstate
        ncl = rows * W
        nj = ncl // 128
        Zp = zo[:, 0]
        Oacc = zo[:, 1]
        outs = outbuf.tile([128, nj, 128], F32, name="outs", tag="outs")
        for kk in range(KK):
            nc.tensor.matmul(Oacc, identb, P2g[kk // KS][:, kk % KS],
                             start=(kk == 0), stop=(kk == KK - 1))

        Rt = work.tile([128, ncl], F32, name="Rt", tag="Rt")
        nc.vector.reciprocal_approx_fast(out=Rt, in_=Zp)
        outn = work.tile([128, ncl], BF16, name="outn", tag="outn")
        nc.vector.tensor_mul(outn, Oacc, Rt)
        Tt = psT.tile([128, ncl], BF16, name="Tt", tag="Tt")
        for j in range(nj):
            nc.tensor.transpose(Tt[:, j * 128:(j + 1) * 128],
                                outn[:, j * 128:(j + 1) * 128], identb)
        nc.scalar.copy(out=outs,
                       in_=Tt.rearrange("p (j d) -> p j d", j=nj))
        dst = out_flat[b][y0 * W:(y0 + rows) * W].rearrange(
            "(j p) d -> p j d", p=128)
        nc.sync.dma_start(out=dst, in_=outs)

    # PE warmup: a chain of dependent matmuls into one scratch PSUM tile
    # keeps the Tensor engine continuously busy through the load phase, so
    # the cost model's p-state ramp (2.4 GHz only after 3us of continuous
    # execution) is already satisfied when the first real matmul issues.
    # The warmup tiles cycle psZ's slots (same 2KB size, same untagged tag)
    # so no extra PSUM bank is needed.
    WARMUP = 24
    ident4 = bass.AP(tensor=identb.tensor, offset=identb.offset,
                     ap=[list(identb.ap[0]), [0, 4], [1, 128]])
    for _ in range(WARMUP):
        warm = psZ.tile([128, 2, NC], F32, name="warm", tag="zo")
        nc.tensor.matmul(warm.rearrange("p a b -> p (a b)"), blockones,
                         ident4, start=True, stop=True)

    # Software pipeline: stage_a of task i+1 is emitted before stage_b of
    # task i; b=1's loads are emitted a few chunks into b=0's stream so
    # their Pool-side descriptor generation doesn't compete at startup.
    DEPTH = 4
    tens = [load_b(0), None]
    # (b, y0, rows): all full 4-row chunks (half-size drain chunks and
    # two-phase loads were tried and measured slower under this scheduler).
    tasks = [(b, ci * R, R) for b in range(BLOC) for ci in range(NCHUNK)]
    pend = []
    for ti, (b, y0, rows) in enumerate(tasks):
        if ti == 14:
            tens[1] = load_b(1)
        Eg = stage_a(tens[b], y0, rows)
        pend.append((b, y0, rows, ti % 2 == 0, Eg))
        while len(pend) > DEPTH:
            pb, py0, prows, pfold, pEg = pend.pop(0)
            stage_b(tens[pb], pb, py0, prows,
                    stage_b1(tens[pb], py0, prows, pfold, pEg))
    for pb, py0, prows, pfold, pEg in pend:
        stage_b(tens[pb], pb, py0, prows,
                stage_b1(tens[pb], py0, prows, pfold, pEg))


_CACHE = {}


def _build():
    if "nc" not in _CACHE:
        nc = bacc.Bacc("TRN2", target_bir_lowering=False, debug=False,
                       num_devices=NCORES)
        q = nc.dram_tensor("q", [BLOC, D, H, W], F32, kind="ExternalInput").ap()
        k = nc.dram_tensor("k", [BLOC, D, H, W], F32, kind="ExternalInput").ap()
        v = nc.dram_tensor("v", [BLOC, D, H, W], F32, kind="ExternalInput").ap()
        out = nc.dram_tensor("out", [BLOC, H, W, D], F32,
                             kind="ExternalOutput").ap()
        with tile.TileContext(nc) as tc:
            with ExitStack() as ctx:
                _body(ctx, tc, out, q, k, v)
        nc.compile()
        _CACHE["nc"] = nc
    return _CACHE["nc"]


def kernel(q, k, v):
    q = np.ascontiguousarray(np.asarray(q), dtype=np.float32)
    k = np.ascontiguousarray(np.asarray(k), dtype=np.float32)
    v = np.ascontiguousarray(np.asarray(v), dtype=np.float32)
    nc = _build()
    in_maps = [
        {
            "q": np.ascontiguousarray(q[i * BLOC:(i + 1) * BLOC]),
            "k": np.ascontiguousarray(k[i * BLOC:(i + 1) * BLOC]),
            "v": np.ascontiguousarray(v[i * BLOC:(i + 1) * BLOC]),
        }
        for i in range(NCORES)
    ]
    res = run_bass_kernel_spmd(nc, in_maps, list(range(NCORES)),
                               trace=PROFILE)
    out = np.concatenate([r["out"] for r in res.results], axis=0)
    if PROFILE:
        kernel.last_exec_time_ns = res.exec_time_ns
        kernel.last_results = res
    return out


if __name__ == "__main__":
    nc = _build()
    print("build OK")
    from concourse.timeline_sim import TimelineSim
    tl = TimelineSim(nc, trace=False)
    t = tl.simulate()
    print(f"TimelineSim: {t/1000.0:.1f} us")



# revision 65
# speedup vs baseline: 1.1049x; 1.0010x over previous
"""DilateAttention Trainium2 Bass kernel.

Problem: q,k,v [16, 128, 64, 64] f32; per-pixel attention over 9 dilated
(dil=2) 3x3 neighbors per head (4 heads x 32 dim); out [16, 64, 64, 128].

Sharding: data-parallel over batch B across 8 cores (2 images/core).

Layout: channel-major ([128 ch partitions, pixels free]). K and V are kept
as zero-padded 68x68 bf16 images so every shifted neighbor view is a
regular (dx, row, col) access pattern; the zero padding reproduces torch
Unfold semantics exactly, including the exp(0) softmax denominator terms
at borders.

Loads: gpsimd (SWDGE) DMAs cast f32->bf16 in flight (1 descriptor per
partition, ~1us Pool descriptor-gen per tensor) — no f32 staging and no
cast ops. k/v land in unpadded bf16 stages and are placed into the padded
images by 4x_2p DVE tensor_copy (0.26 ns/elem). A chain of dummy matmuls
keeps the PE continuously busy through the load phase so the cost model's
p-state ramp (2.4 GHz needs 3us of continuous execution) is already
satisfied when the first real matmul issues.

Per 4-row chunk (256 px), kk grouped by dy (3 groups of 3), software
pipelined four chunks deep:
  products Q*K_kk (DVE bf16 2x_1p; tail planes on GpSimd) -> per-head
  sums via PE block-ones matmuls into grouped PSUM tiles -> exp on ACT
  (PSUM -> SBUF bf16, one op per group; full 128-partition extent so the
  per-head -> per-channel broadcast is free) -> denominator via 9
  accumulated PE (block-ones/32) matmuls -> AV products (DVE + GpSimd
  tail, alternating split to balance) -> sum over kk via accumulated PE
  identity matmuls -> reciprocal_approx_fast + normalize to bf16 (DVE)
  -> PE bf16 transpose (1 cyc/row) to pixel-major -> ACT copy casts to
  f32 SBUF -> per-chunk output DMA.

Engine busy (cost-model sim, per core): PE ~96us (the steady-state bound:
scores 960 + Z 960 + O 960 + transpose 107 ns per 256-px chunk), DVE
~99us, ACT ~93us, Pool ~92us; wall 115.3us = PE + ~11us startup + ~6us
drain. (5-dim-AP merged product ops shaved this further in the cost
model but the walrus backend rejects them, so per-dy ops stand. Also
measured slower: two-phase b0 loads, half-size drain chunks, early
all-DVE products, f32 bootstrap loads, depth tapering.)
"""

import numpy as np
from contextlib import ExitStack

import concourse.bass as bass
import concourse.bacc as bacc
import concourse.tile as tile
from concourse import mybir
from concourse.bass_utils import run_bass_kernel_spmd
from concourse.masks import make_identity

F32 = mybir.dt.float32
BF16 = mybir.dt.bfloat16

B, D, H, W = 16, 128, 64, 64
NCORES = 8
BLOC = B // NCORES          # images per core
HEADS, HD = 4, 32
KS, DIL, PAD = 3, 2, 2
HP = H + 2 * PAD            # 68 (y-padded)
KK = KS * KS                # 9
SCALE = float(HD) ** -0.5
R = 4                       # image rows per chunk
NC = R * W                  # 256 pixels per chunk
NCHUNK = H // R             # 16
OGRP = 1                    # chunks batched per output DMA

PROFILE = False


WP2 = W + 2 * PAD  # 68 (x-padded too)


def _build_padded(nc, dst, stage, cast_engine):
    """dst: [128, HP, WP2] bf16 zero-padded image; stage: [128, H, W] f32."""
    nc.gpsimd.memset(dst[:, 0:PAD, :], 0.0)
    nc.gpsimd.memset(dst[:, HP - PAD:HP, :], 0.0)
    nc.gpsimd.memset(dst[:, PAD:HP - PAD, 0:PAD], 0.0)
    nc.gpsimd.memset(dst[:, PAD:HP - PAD, WP2 - PAD:WP2], 0.0)
    if cast_engine == "act":
        nc.scalar.copy(out=dst[:, PAD:HP - PAD, PAD:WP2 - PAD], in_=stage)
    elif cast_engine == "pool":
        nc.gpsimd.tensor_copy(out=dst[:, PAD:HP - PAD, PAD:WP2 - PAD],
                              in_=stage)
    else:
        nc.vector.tensor_copy(out=dst[:, PAD:HP - PAD, PAD:WP2 - PAD],
                              in_=stage)


USE_STT = False


def _pool_mul(nc, out, a, b):
    """Pool-engine multiply via scalar_tensor_tensor: the TensorScalarPtr
    opcode takes the 0.60 gpsimd efficiency bucket instead of
    tensor_tensor's 0.42 Multiply bucket."""
    if USE_STT:
        nc.gpsimd.scalar_tensor_tensor(out, a, 1.0, b,
                                       op0=mybir.AluOpType.mult,
                                       op1=mybir.AluOpType.mult)
    else:
        nc.gpsimd.tensor_mul(out, a, b)


def _pool_add(nc, out, a, b):
    if USE_STT:
        nc.gpsimd.scalar_tensor_tensor(out, a, 1.0, b,
                                       op0=mybir.AluOpType.mult,
                                       op1=mybir.AluOpType.add)
    else:
        nc.gpsimd.tensor_add(out, a, b)


def _bcast_q(qbf, y0, rows=R):
    """[128, 3, rows, W] view of qbf rows y0.. with a 0-step dx axis."""
    base = qbf[:, y0:y0 + rows, :]
    return bass.AP(
        tensor=base.tensor,
        offset=base.offset,
        ap=[list(base.ap[0]), [0, KS], [W, rows], [1, W]],
    )


def _shift_view(pad_t, y0, idy, i0=0, n=KS, rows=R):
    """[128, n(idx), rows, W] view of padded image at dy=idy for chunk y0,
    idx range [i0, i0+n)."""
    return bass.AP(
        tensor=pad_t.tensor,
        offset=pad_t.offset + (y0 + DIL * idy) * WP2 + DIL * i0,
        ap=[list(pad_t.ap[0]), [DIL, n], [WP2, rows], [1, W]],
    )


def _body(ctx: ExitStack, tc: tile.TileContext, out_ap, q_ap, k_ap, v_ap):
    nc = tc.nc

    consts = ctx.enter_context(tc.tile_pool(name="consts", bufs=1))
    stage_pool = ctx.enter_context(tc.tile_pool(name="stage_pool", bufs=3))
    perb = ctx.enter_context(tc.tile_pool(name="perb", bufs=2))
    work = ctx.enter_context(tc.tile_pool(name="work", bufs=8))
    egrp = ctx.enter_context(tc.tile_pool(name="egrp", bufs=14))
    outbuf = ctx.enter_context(tc.tile_pool(name="outbuf", bufs=6))
    psS = ctx.enter_context(tc.tile_pool(name="psS", bufs=2, space="PSUM"))
    psZ = ctx.enter_context(tc.tile_pool(name="psZ", bufs=2, space="PSUM"))
    psT = ctx.enter_context(tc.tile_pool(name="psT", bufs=2, space="PSUM"))

    # Constant stationary matrices.
    blockones = consts.tile([128, 128], BF16)   # 1 if same head
    bo32 = consts.tile([128, 128], BF16)        # 1/32 if same head
    identb = consts.tile([128, 128], BF16)
    identf = consts.tile([128, 128], F32)
    nc.vector.memset(blockones, 0.0)
    nc.vector.memset(bo32, 0.0)
    for h in range(HEADS):
        s = slice(h * HD, (h + 1) * HD)
        nc.vector.memset(blockones[s, s], 1.0)
        nc.vector.memset(bo32[s, s], 1.0 / HD)
    make_identity(nc, identb)
    make_identity(nc, identf)

    qf = q_ap.rearrange("b d h w -> b d (h w)")
    out_flat = out_ap.rearrange("b h w d -> b (h w) d")

    NQ = 2          # pad-copy row bands per image
    RB = H // NQ    # rows per band

    def load_b(b):
        # gpsimd (SWDGE) DMAs cast f32->bf16 in flight: one descriptor per
        # partition, ~1us of Pool descriptor-gen per tensor, and no f32
        # staging or cast ops at all. k/v land in unpadded bf16 stages and
        # are placed into the zero-padded images by 4x_2p DVE copies.
        # For b=0 the q/k transfers are split and interleaved so the first
        # chunks' working set lands in ~4us instead of ~7.
        qbf = perb.tile([128, H, W], BF16, name="qbf")
        kst = stage_pool.tile([128, H, W], BF16, name="kst", tag="stage")
        vst = stage_pool.tile([128, H, W], BF16, name="vst", tag="stage")
        kpad = perb.tile([128, HP, WP2], BF16, name="kpad")
        vpad = perb.tile([128, HP, WP2], BF16, name="vpad")
        # For b=0, the first q/k half-gens go ahead of everything so their
        # transfers start immediately; the memsets fill Pool's time while
        # those transfers are in flight, then the remaining gens follow.
        qsrc0 = qf[b].rearrange("d (h w) -> d h w", w=W)
        if b == 0:
            nc.gpsimd.dma_start(out=qbf[:, 0:H // 2], in_=qsrc0[:, 0:H // 2])
            nc.gpsimd.dma_start(out=kst[:, 0:H // 2],
                                in_=k_ap[b][:, 0:H // 2])
        for pad_t in (kpad, vpad):
            nc.gpsimd.memset(pad_t[:, 0:PAD, :], 0.0)
            nc.gpsimd.memset(pad_t[:, HP - PAD:HP, :], 0.0)
            nc.gpsimd.memset(pad_t[:, PAD:HP - PAD, 0:PAD], 0.0)
            nc.gpsimd.memset(pad_t[:, PAD:HP - PAD, WP2 - PAD:WP2], 0.0)
        qsrc = qf[b].rearrange("d (h w) -> d h w", w=W)
        if b == 0:
            hh = H // 2
            nc.gpsimd.dma_start(out=qbf[:, hh:H], in_=qsrc[:, hh:H])
            nc.gpsimd.dma_start(out=kst[:, hh:H], in_=k_ap[b][:, hh:H])
        else:
            nc.gpsimd.dma_start(out=qbf, in_=qsrc)
            nc.gpsimd.dma_start(out=kst, in_=k_ap[b])
        nc.gpsimd.dma_start(out=vst, in_=v_ap[b])
        for pad_t, st in ((kpad, kst), (vpad, vst)):
            for i in range(NQ):
                rs = slice(i * RB, (i + 1) * RB)
                ps = slice(PAD + i * RB, PAD + (i + 1) * RB)
                nc.vector.tensor_copy(out=pad_t[:, ps, PAD:WP2 - PAD],
                                      in_=st[:, rs, :])
        return qbf, kpad, vpad

    def stage_a(tens, y0, rows):
        """QK products -> per-head score matmuls -> exp. Returns E groups."""
        qbf, kpad, vpad = tens
        qv = _bcast_q(qbf, y0, rows)
        Eg = []
        for g in range(KS):  # g == idy
            Pg = egrp.tile([128, KS, rows, W], BF16, name="Pg", tag="Pg")
            if g == KS - 1:
                nc.vector.tensor_mul(Pg[:, 0:1], qv[:, 0:1],
                                     _shift_view(kpad, y0, g, 0, 1, rows))
                _pool_mul(nc, Pg[:, 1:3], qv[:, 1:3],
                          _shift_view(kpad, y0, g, 1, 2, rows))
            else:
                nc.vector.tensor_mul(Pg, qv, _shift_view(kpad, y0, g,
                                                         rows=rows))
            Sgt = psS.tile([128, KS, rows, W], F32, name="Sgt", tag="Sgt")
            # Planes 0-1 are one bank-aligned 2KB PSUM window: one N=512 MM.
            nc.tensor.matmul(Sgt[:, 0:2], blockones, Pg[:, 0:2],
                             start=True, stop=True)
            nc.tensor.matmul(Sgt[:, 2], blockones, Pg[:, 2],
                             start=True, stop=True)
            Egt = egrp.tile([128, KS, rows, W], BF16, name="Egt", tag="Egt")
            nc.scalar.activation(out=Egt, in_=Sgt,
                                 func=mybir.ActivationFunctionType.Exp,
                                 scale=SCALE)
            Eg.append(Egt)
        return Eg

    state = {"outs": None}

    def stage_b1(tens, y0, rows, fold, Eg):
        """Z-sum and AV products."""
        qbf, kpad, vpad = tens
        zo = psZ.tile([128, 2, rows * W], F32, name="zo", tag="zo")
        Zp = zo[:, 0]
        for kk in range(KK):
            nc.tensor.matmul(Zp, bo32, Eg[kk // KS][:, kk % KS],
                             start=(kk == 0), stop=(kk == KK - 1))
        P2g = []
        for g in range(KS):
            P2t = egrp.tile([128, KS, rows, W], BF16, name="P2t", tag="P2t")
            if g == KS - 1:
                # Last group's tail planes are consumed last by the Oacc
                # accumulation; run them on the GpSimd engine. On alternate
                # chunks the whole group goes to GpSimd to balance DVE.
                if fold:
                    _pool_mul(nc, P2t, Eg[g],
                              _shift_view(vpad, y0, g, rows=rows))
                else:
                    nc.vector.tensor_mul(P2t[:, 0:1], Eg[g][:, 0:1],
                                         _shift_view(vpad, y0, g, 0, 1,
                                                     rows))
                    _pool_mul(nc, P2t[:, 1:3], Eg[g][:, 1:3],
                              _shift_view(vpad, y0, g, 1, 2, rows))
            else:
                nc.vector.tensor_mul(P2t, Eg[g],
                                     _shift_view(vpad, y0, g, rows=rows))
            P2g.append(P2t)
        return zo, P2g

    def stage_b(tens, b, y0, rows, bstate):
        """kk-sum, normalize, transpose, store."""
        zo, P2g = bstate
        ncl = rows * W
        nj = ncl // 128
        Zp = zo[:, 0]
        Oacc = zo[:, 1]
        outs = outbuf.tile([128, nj, 128], F32, name="outs", tag="outs")
        for kk in range(KK):
            nc.tensor.matmul(Oacc, identb, P2g[kk // KS][:, kk % KS],
                             start=(kk == 0), stop=(kk == KK - 1))

        Rt = work.tile([128, ncl], F32, name="Rt", tag="Rt")
        nc.vector.reciprocal_approx_fast(out=Rt, in_=Zp)
        outn = work.tile([128, ncl], BF16, name="outn", tag="outn")
        nc.vector.tensor_mul(outn, Oacc, Rt)
        Tt = psT.tile([128, ncl], BF16, name="Tt", tag="Tt")
        for j in range(nj):
            nc.tensor.transpose(Tt[:, j * 128:(j + 1) * 128],
                                outn[:, j * 128:(j + 1) * 128], identb)
        nc.scalar.copy(out=outs,
                       in_=Tt.rearrange("p (j d) -> p j d", j=nj))
        dst = out_flat[b][y0 * W:(y0 + rows) * W].rearrange(
            "(j p) d -> p j d", p=128)
        nc.sync.dma_start(out=dst, in_=outs)

    # PE warmup: a chain of dependent matmuls into one scratch PSUM tile
    # keeps the Tensor engine continuously busy through the load phase, so
    # the cost model's p-state ramp (2.4 GHz only after 3us of continuous
    # execution) is already satisfied when the first real matmul issues.
    # The warmup tiles cycle psZ's slots (same 2KB size, same untagged tag)
    # so no extra PSUM bank is needed.
    WARMUP = 24
    ident4 = bass.AP(tensor=identb.tensor, offset=identb.offset,
                     ap=[list(identb.ap[0]), [0, 4], [1, 128]])
    for _ in range(WARMUP):
        warm = psZ.tile([128, 2, NC], F32, name="warm", tag="zo")
        nc.tensor.matmul(warm.rearrange("p a b -> p (a b)"), blockones,
                         ident4, start=True, stop=True)

    # Software pipeline: stage_a of task i+1 is emitted before stage_b of
    # task i; b=1's loads are emitted a few chunks into b=0's stream so
    # their Pool-side descriptor generation doesn't compete at startup.
    DEPTH = 4
    tens = [load_b(0), None]
    # (b, y0, rows): all full 4-row chunks (half-size drain chunks and
    # two-phase loads were tried and measured slower under this scheduler).
    tasks = [(b, ci * R, R) for b in range(BLOC) for ci in range(NCHUNK)]
    pend = []
    for ti, (b, y0, rows) in enumerate(tasks):
        if ti == 14:
            tens[1] = load_b(1)
        Eg = stage_a(tens[b], y0, rows)
        pend.append((b, y0, rows, ti % 2 == 0, Eg))
        while len(pend) > DEPTH:
            pb, py0, prows, pfold, pEg = pend.pop(0)
            stage_b(tens[pb], pb, py0, prows,
                    stage_b1(tens[pb], py0, prows, pfold, pEg))
    for pb, py0, prows, pfold, pEg in pend:
        stage_b(tens[pb], pb, py0, prows,
                stage_b1(tens[pb], py0, prows, pfold, pEg))


_CACHE = {}


def _build():
    if "nc" not in _CACHE:
        nc = bacc.Bacc("TRN2", target_bir_lowering=False, debug=False,
                       num_devices=NCORES)
        q = nc.dram_tensor("q", [BLOC, D, H, W], F32, kind="ExternalInput").ap()
        k = nc.dram_tensor("k", [BLOC, D, H, W], F32, kind="ExternalInput").ap()
        v = nc.dram_tensor("v", [BLOC, D, H, W], F32, kind="ExternalInput").ap()
        out = nc.dram_tensor("out", [BLOC, H, W, D], F32,
                             kind="ExternalOutput").ap()
        with tile.TileContext(nc) as tc:
            with ExitStack() as ctx:
                _body(ctx, tc, out, q, k, v)
        nc.compile()
        _CACHE["nc"] = nc
    return _CACHE["nc"]


def kernel(q, k, v):
    q = np.ascontiguousarray(np.asarray(q), dtype=np.float32)
    k = np.ascontiguousarray(np.asarray(k), dtype=np.float32)
    v = np.ascontiguousarray(np.asarray(v), dtype=np.float32)
    nc = _build()
    in_maps = [
        {
            "q": np.ascontiguousarray(q[i * BLOC:(i + 1) * BLOC]),
            "k": np.ascontiguousarray(k[i * BLOC:(i + 1) * BLOC]),
            "v": np.ascontiguousarray(v[i * BLOC:(i + 1) * BLOC]),
        }
        for i in range(NCORES)
    ]
    res = run_bass_kernel_spmd(nc, in_maps, list(range(NCORES)),
                               trace=PROFILE)
    out = np.concatenate([r["out"] for r in res.results], axis=0)
    if PROFILE:
        kernel.last_exec_time_ns = res.exec_time_ns
        kernel.last_results = res
    return out


if __name__ == "__main__":
    nc = _build()
    print("build OK")
    from concourse.timeline_sim import TimelineSim
    tl = TimelineSim(nc, trace=False)
    t = tl.simulate()
    print(f"TimelineSim: {t/1000.0:.1f} us")



# revision 67
# speedup vs baseline: 1.1103x; 1.0049x over previous
"""DilateAttention Trainium2 Bass kernel.

Problem: q,k,v [16, 128, 64, 64] f32; per-pixel attention over 9 dilated
(dil=2) 3x3 neighbors per head (4 heads x 32 dim); out [16, 64, 64, 128].

Sharding: data-parallel over batch B across 8 cores (2 images/core).

Layout: channel-major ([128 ch partitions, pixels free]). K and V are kept
as zero-padded 68x68 bf16 images so every shifted neighbor view is a
regular (dx, row, col) access pattern; the zero padding reproduces torch
Unfold semantics exactly, including the exp(0) softmax denominator terms
at borders.

Loads: gpsimd (SWDGE) DMAs cast f32->bf16 in flight (1 descriptor per
partition, ~1us Pool descriptor-gen per tensor) — no f32 staging and no
cast ops. k/v land in unpadded bf16 stages and are placed into the padded
images by 4x_2p DVE tensor_copy (0.26 ns/elem). A chain of dummy matmuls
keeps the PE continuously busy through the load phase so the cost model's
p-state ramp (2.4 GHz needs 3us of continuous execution) is already
satisfied when the first real matmul issues.

Per 4-row chunk (256 px), kk grouped by dy (3 groups of 3), software
pipelined four chunks deep:
  products Q*K_kk (DVE bf16 2x_1p; tail planes on GpSimd) -> per-head
  sums via PE block-ones matmuls into grouped PSUM tiles -> exp on ACT
  (PSUM -> SBUF bf16, one op per group; full 128-partition extent so the
  per-head -> per-channel broadcast is free) -> denominator via 9
  accumulated PE (block-ones/32) matmuls -> AV products (DVE + GpSimd
  tail, alternating split to balance) -> sum over kk via accumulated PE
  identity matmuls -> reciprocal_approx_fast + normalize to bf16 (DVE)
  -> PE bf16 transpose (1 cyc/row) to pixel-major -> ACT copy casts to
  f32 SBUF -> per-chunk output DMA.

Engine busy (cost-model sim, per core): PE ~96us (the steady-state bound:
scores 960 + Z 960 + O 960 + transpose 107 ns per 256-px chunk), DVE
~99us, ACT ~93us, Pool ~92us; wall 115.2us = PE + ~11us startup + ~6us
drain. (5-dim-AP merged product ops shaved this further in the cost
model but the walrus backend rejects them, so per-dy ops stand. Also
measured slower: two-phase b0 loads, half-size drain chunks, early
all-DVE products, f32 bootstrap loads, depth tapering.)
"""

import numpy as np
from contextlib import ExitStack

import concourse.bass as bass
import concourse.bacc as bacc
import concourse.tile as tile
from concourse import mybir
from concourse.bass_utils import run_bass_kernel_spmd
from concourse.masks import make_identity

F32 = mybir.dt.float32
BF16 = mybir.dt.bfloat16

B, D, H, W = 16, 128, 64, 64
NCORES = 8
BLOC = B // NCORES          # images per core
HEADS, HD = 4, 32
KS, DIL, PAD = 3, 2, 2
HP = H + 2 * PAD            # 68 (y-padded)
KK = KS * KS                # 9
SCALE = float(HD) ** -0.5
R = 4                       # image rows per chunk
NC = R * W                  # 256 pixels per chunk
NCHUNK = H // R             # 16
OGRP = 1                    # chunks batched per output DMA

PROFILE = False


WP2 = W + 2 * PAD  # 68 (x-padded too)


def _build_padded(nc, dst, stage, cast_engine):
    """dst: [128, HP, WP2] bf16 zero-padded image; stage: [128, H, W] f32."""
    nc.gpsimd.memset(dst[:, 0:PAD, :], 0.0)
    nc.gpsimd.memset(dst[:, HP - PAD:HP, :], 0.0)
    nc.gpsimd.memset(dst[:, PAD:HP - PAD, 0:PAD], 0.0)
    nc.gpsimd.memset(dst[:, PAD:HP - PAD, WP2 - PAD:WP2], 0.0)
    if cast_engine == "act":
        nc.scalar.copy(out=dst[:, PAD:HP - PAD, PAD:WP2 - PAD], in_=stage)
    elif cast_engine == "pool":
        nc.gpsimd.tensor_copy(out=dst[:, PAD:HP - PAD, PAD:WP2 - PAD],
                              in_=stage)
    else:
        nc.vector.tensor_copy(out=dst[:, PAD:HP - PAD, PAD:WP2 - PAD],
                              in_=stage)


USE_STT = False


def _pool_mul(nc, out, a, b):
    """Pool-engine multiply via scalar_tensor_tensor: the TensorScalarPtr
    opcode takes the 0.60 gpsimd efficiency bucket instead of
    tensor_tensor's 0.42 Multiply bucket."""
    if USE_STT:
        nc.gpsimd.scalar_tensor_tensor(out, a, 1.0, b,
                                       op0=mybir.AluOpType.mult,
                                       op1=mybir.AluOpType.mult)
    else:
        nc.gpsimd.tensor_mul(out, a, b)


def _pool_add(nc, out, a, b):
    if USE_STT:
        nc.gpsimd.scalar_tensor_tensor(out, a, 1.0, b,
                                       op0=mybir.AluOpType.mult,
                                       op1=mybir.AluOpType.add)
    else:
        nc.gpsimd.tensor_add(out, a, b)


def _bcast_q(qbf, y0, rows=R):
    """[128, 3, rows, W] view of qbf rows y0.. with a 0-step dx axis."""
    base = qbf[:, y0:y0 + rows, :]
    return bass.AP(
        tensor=base.tensor,
        offset=base.offset,
        ap=[list(base.ap[0]), [0, KS], [W, rows], [1, W]],
    )


def _shift_view(pad_t, y0, idy, i0=0, n=KS, rows=R):
    """[128, n(idx), rows, W] view of padded image at dy=idy for chunk y0,
    idx range [i0, i0+n)."""
    return bass.AP(
        tensor=pad_t.tensor,
        offset=pad_t.offset + (y0 + DIL * idy) * WP2 + DIL * i0,
        ap=[list(pad_t.ap[0]), [DIL, n], [WP2, rows], [1, W]],
    )


def _body(ctx: ExitStack, tc: tile.TileContext, out_ap, q_ap, k_ap, v_ap):
    nc = tc.nc

    consts = ctx.enter_context(tc.tile_pool(name="consts", bufs=1))
    stage_pool = ctx.enter_context(tc.tile_pool(name="stage_pool", bufs=3))
    perb = ctx.enter_context(tc.tile_pool(name="perb", bufs=2))
    work = ctx.enter_context(tc.tile_pool(name="work", bufs=8))
    egrp = ctx.enter_context(tc.tile_pool(name="egrp", bufs=14))
    outbuf = ctx.enter_context(tc.tile_pool(name="outbuf", bufs=6))
    psS = ctx.enter_context(tc.tile_pool(name="psS", bufs=2, space="PSUM"))
    psZ = ctx.enter_context(tc.tile_pool(name="psZ", bufs=2, space="PSUM"))
    psT = ctx.enter_context(tc.tile_pool(name="psT", bufs=2, space="PSUM"))

    # Constant stationary matrices.
    blockones = consts.tile([128, 128], BF16)   # 1 if same head
    bo32 = consts.tile([128, 128], BF16)        # 1/32 if same head
    identb = consts.tile([128, 128], BF16)
    identf = consts.tile([128, 128], F32)
    nc.vector.memset(blockones, 0.0)
    nc.vector.memset(bo32, 0.0)
    for h in range(HEADS):
        s = slice(h * HD, (h + 1) * HD)
        nc.vector.memset(blockones[s, s], 1.0)
        nc.vector.memset(bo32[s, s], 1.0 / HD)
    make_identity(nc, identb)
    make_identity(nc, identf)

    qf = q_ap.rearrange("b d h w -> b d (h w)")
    out_flat = out_ap.rearrange("b h w d -> b (h w) d")

    NQ = 2          # pad-copy row bands per image
    RB = H // NQ    # rows per band

    def load_b(b):
        # gpsimd (SWDGE) DMAs cast f32->bf16 in flight: one descriptor per
        # partition, ~1us of Pool descriptor-gen per tensor, and no f32
        # staging or cast ops at all. k/v land in unpadded bf16 stages and
        # are placed into the zero-padded images by 4x_2p DVE copies.
        # For b=0 the q/k transfers are split and interleaved so the first
        # chunks' working set lands in ~4us instead of ~7.
        qbf = perb.tile([128, H, W], BF16, name="qbf")
        kst = stage_pool.tile([128, H, W], BF16, name="kst", tag="stage")
        vst = stage_pool.tile([128, H, W], BF16, name="vst", tag="stage")
        kpad = perb.tile([128, HP, WP2], BF16, name="kpad")
        vpad = perb.tile([128, HP, WP2], BF16, name="vpad")
        # For b=0, the first q/k half-gens go ahead of everything so their
        # transfers start immediately; the memsets fill Pool's time while
        # those transfers are in flight, then the remaining gens follow.
        qsrc0 = qf[b].rearrange("d (h w) -> d h w", w=W)
        if b == 0:
            nc.gpsimd.dma_start(out=qbf[:, 0:H // 2], in_=qsrc0[:, 0:H // 2])
            nc.gpsimd.dma_start(out=kst[:, 0:H // 2],
                                in_=k_ap[b][:, 0:H // 2])
        for pad_t in (kpad, vpad):
            nc.gpsimd.memset(pad_t[:, 0:PAD, :], 0.0)
            nc.gpsimd.memset(pad_t[:, HP - PAD:HP, :], 0.0)
            nc.gpsimd.memset(pad_t[:, PAD:HP - PAD, 0:PAD], 0.0)
            nc.gpsimd.memset(pad_t[:, PAD:HP - PAD, WP2 - PAD:WP2], 0.0)
        qsrc = qf[b].rearrange("d (h w) -> d h w", w=W)
        if b == 0:
            hh = H // 2
            nc.gpsimd.dma_start(out=qbf[:, hh:H], in_=qsrc[:, hh:H])
            nc.gpsimd.dma_start(out=kst[:, hh:H], in_=k_ap[b][:, hh:H])
        else:
            nc.gpsimd.dma_start(out=qbf, in_=qsrc)
            nc.gpsimd.dma_start(out=kst, in_=k_ap[b])
        nc.gpsimd.dma_start(out=vst, in_=v_ap[b])
        for pad_t, st in ((kpad, kst), (vpad, vst)):
            for i in range(NQ):
                rs = slice(i * RB, (i + 1) * RB)
                ps = slice(PAD + i * RB, PAD + (i + 1) * RB)
                nc.vector.tensor_copy(out=pad_t[:, ps, PAD:WP2 - PAD],
                                      in_=st[:, rs, :])
        return qbf, kpad, vpad

    def stage_a(tens, y0, rows):
        """QK products -> per-head score matmuls -> exp. Returns E groups."""
        qbf, kpad, vpad = tens
        qv = _bcast_q(qbf, y0, rows)
        Eg = []
        for g in range(KS):  # g == idy
            Pg = egrp.tile([128, KS, rows, W], BF16, name="Pg", tag="Pg")
            if g == KS - 1:
                nc.vector.tensor_mul(Pg[:, 0:1], qv[:, 0:1],
                                     _shift_view(kpad, y0, g, 0, 1, rows))
                _pool_mul(nc, Pg[:, 1:3], qv[:, 1:3],
                          _shift_view(kpad, y0, g, 1, 2, rows))
            else:
                nc.vector.tensor_mul(Pg, qv, _shift_view(kpad, y0, g,
                                                         rows=rows))
            Sgt = psS.tile([128, KS, rows, W], F32, name="Sgt", tag="Sgt")
            # Planes 0-1 are one bank-aligned 2KB PSUM window: one N=512 MM.
            nc.tensor.matmul(Sgt[:, 0:2], blockones, Pg[:, 0:2],
                             start=True, stop=True)
            nc.tensor.matmul(Sgt[:, 2], blockones, Pg[:, 2],
                             start=True, stop=True)
            Egt = egrp.tile([128, KS, rows, W], BF16, name="Egt", tag="Egt")
            nc.scalar.activation(out=Egt, in_=Sgt,
                                 func=mybir.ActivationFunctionType.Exp,
                                 scale=SCALE)
            Eg.append(Egt)
        return Eg

    state = {"outs": None}

    def stage_b1(tens, y0, rows, fold, Eg):
        """Z-sum and AV products."""
        qbf, kpad, vpad = tens
        zo = psZ.tile([128, 2, rows * W], F32, name="zo", tag="zo")
        Zp = zo[:, 0]
        for kk in range(KK):
            nc.tensor.matmul(Zp, bo32, Eg[kk // KS][:, kk % KS],
                             start=(kk == 0), stop=(kk == KK - 1))
        P2g = []
        for g in range(KS):
            P2t = egrp.tile([128, KS, rows, W], BF16, name="P2t", tag="P2t")
            if g == KS - 1:
                # Last group's tail planes are consumed last by the Oacc
                # accumulation; run them on the GpSimd engine. On alternate
                # chunks the whole group goes to GpSimd to balance DVE.
                if fold:
                    _pool_mul(nc, P2t, Eg[g],
                              _shift_view(vpad, y0, g, rows=rows))
                else:
                    nc.vector.tensor_mul(P2t[:, 0:1], Eg[g][:, 0:1],
                                         _shift_view(vpad, y0, g, 0, 1,
                                                     rows))
                    _pool_mul(nc, P2t[:, 1:3], Eg[g][:, 1:3],
                              _shift_view(vpad, y0, g, 1, 2, rows))
            else:
                nc.vector.tensor_mul(P2t, Eg[g],
                                     _shift_view(vpad, y0, g, rows=rows))
            P2g.append(P2t)
        return zo, P2g

    def stage_b(tens, b, y0, rows, bstate):
        """kk-sum, normalize, transpose, store."""
        zo, P2g = bstate
        ncl = rows * W
        nj = ncl // 128
        Zp = zo[:, 0]
        Oacc = zo[:, 1]
        outs = outbuf.tile([128, nj, 128], F32, name="outs", tag="outs")
        for kk in range(KK):
            nc.tensor.matmul(Oacc, identb, P2g[kk // KS][:, kk % KS],
                             start=(kk == 0), stop=(kk == KK - 1))

        Rt = work.tile([128, ncl], F32, name="Rt", tag="Rt")
        nc.vector.reciprocal_approx_fast(out=Rt, in_=Zp)
        outn = work.tile([128, ncl], BF16, name="outn", tag="outn")
        nc.vector.tensor_mul(outn, Oacc, Rt)
        Tt = psT.tile([128, ncl], BF16, name="Tt", tag="Tt")
        for j in range(nj):
            nc.tensor.transpose(Tt[:, j * 128:(j + 1) * 128],
                                outn[:, j * 128:(j + 1) * 128], identb)
        nc.scalar.copy(out=outs,
                       in_=Tt.rearrange("p (j d) -> p j d", j=nj))
        dst = out_flat[b][y0 * W:(y0 + rows) * W].rearrange(
            "(j p) d -> p j d", p=128)
        nc.sync.dma_start(out=dst, in_=outs)

    # PE warmup: a chain of dependent matmuls into one scratch PSUM tile
    # keeps the Tensor engine continuously busy through the load phase, so
    # the cost model's p-state ramp (2.4 GHz only after 3us of continuous
    # execution) is already satisfied when the first real matmul issues.
    # The warmup tiles cycle psZ's slots (same 2KB size, same untagged tag)
    # so no extra PSUM bank is needed.
    WARMUP = 24
    ident4 = bass.AP(tensor=identb.tensor, offset=identb.offset,
                     ap=[list(identb.ap[0]), [0, 4], [1, 128]])
    for _ in range(WARMUP):
        warm = psZ.tile([128, 2, NC], F32, name="warm", tag="zo")
        nc.tensor.matmul(warm.rearrange("p a b -> p (a b)"), blockones,
                         ident4, start=True, stop=True)

    # Software pipeline: stage_a of task i+1 is emitted before stage_b of
    # task i; b=1's loads are emitted a few chunks into b=0's stream so
    # their Pool-side descriptor generation doesn't compete at startup.
    DEPTH = 4
    tens = [load_b(0), None]
    # (b, y0, rows): all full 4-row chunks (half-size drain chunks and
    # two-phase loads were tried and measured slower under this scheduler).
    tasks = [(b, ci * R, R) for b in range(BLOC) for ci in range(NCHUNK)]
    pend = []
    for ti, (b, y0, rows) in enumerate(tasks):
        if ti == 14:
            tens[1] = load_b(1)
        Eg = stage_a(tens[b], y0, rows)
        pend.append((b, y0, rows, ti % 4 != 0, Eg))
        while len(pend) > DEPTH:
            pb, py0, prows, pfold, pEg = pend.pop(0)
            stage_b(tens[pb], pb, py0, prows,
                    stage_b1(tens[pb], py0, prows, pfold, pEg))
    for pb, py0, prows, pfold, pEg in pend:
        stage_b(tens[pb], pb, py0, prows,
                stage_b1(tens[pb], py0, prows, pfold, pEg))


_CACHE = {}


def _build():
    if "nc" not in _CACHE:
        nc = bacc.Bacc("TRN2", target_bir_lowering=False, debug=False,
                       num_devices=NCORES)
        q = nc.dram_tensor("q", [BLOC, D, H, W], F32, kind="ExternalInput").ap()
        k = nc.dram_tensor("k", [BLOC, D, H, W], F32, kind="ExternalInput").ap()
        v = nc.dram_tensor("v", [BLOC, D, H, W], F32, kind="ExternalInput").ap()
        out = nc.dram_tensor("out", [BLOC, H, W, D], F32,
                             kind="ExternalOutput").ap()
        with tile.TileContext(nc) as tc:
            with ExitStack() as ctx:
                _body(ctx, tc, out, q, k, v)
        nc.compile()
        _CACHE["nc"] = nc
    return _CACHE["nc"]


def kernel(q, k, v):
    q = np.ascontiguousarray(np.asarray(q), dtype=np.float32)
    k = np.ascontiguousarray(np.asarray(k), dtype=np.float32)
    v = np.ascontiguousarray(np.asarray(v), dtype=np.float32)
    nc = _build()
    in_maps = [
        {
            "q": np.ascontiguousarray(q[i * BLOC:(i + 1) * BLOC]),
            "k": np.ascontiguousarray(k[i * BLOC:(i + 1) * BLOC]),
            "v": np.ascontiguousarray(v[i * BLOC:(i + 1) * BLOC]),
        }
        for i in range(NCORES)
    ]
    res = run_bass_kernel_spmd(nc, in_maps, list(range(NCORES)),
                               trace=PROFILE)
    out = np.concatenate([r["out"] for r in res.results], axis=0)
    if PROFILE:
        kernel.last_exec_time_ns = res.exec_time_ns
        kernel.last_results = res
    return out


if __name__ == "__main__":
    nc = _build()
    print("build OK")
    from concourse.timeline_sim import TimelineSim
    tl = TimelineSim(nc, trace=False)
    t = tl.simulate()
    print(f"TimelineSim: {t/1000.0:.1f} us")

